# revision 33
# baseline (speedup 1.0000x reference)
"""Block-diagonal grouped GEMM (BlockDense) for Trainium2, 8 NeuronCores.

Problem: x:(8192, 16384) f32, W:(1024, 16, 16) f32
         out[b, g*16+h] = relu(sum_w x[b, g*16+w] * W[g, w, h])

Strategy:
  - Data-parallel shard of the batch dim across 8 cores (1024 rows each).
  - Host relayouts each x shard so features sit on SBUF partitions
    (the PE contracts along partitions); 8 groups are packed into one
    128x128 block-diagonal weight supergroup so the full PE array is used.
  - Per core: for each of 16 column blocks (1024 cols = 8 supergroups):
    DMA x-block + W-block, 64 fp32 matmuls (stationary = xT tile),
    relu PSUM->SBUF on alternating Scalar/Vector engines, DMA out.
"""

import sys

import numpy as np

import concourse.bass as bass
import concourse.mybir as mybir
import concourse.tile as tile
from concourse import bacc, bass_utils


def _ensure_axon_hooks_shim():
    """The bare agent image lacks antenv.axon_hooks; bass_utils imports it
    when trace=True under axon. Provide a working shim (ctypes NTFF hook if
    the axon .so supports it, else None -> tracing is skipped gracefully)."""
    try:
        import antenv.axon_hooks  # noqa: F401
        return
    except ImportError:
        pass
    import types

    hook = None
    try:
        from trn_agent_boot.trn_boot import _ntff_profile_via_ctypes

        hook = _ntff_profile_via_ctypes("/opt/axon/libaxon_pjrt.so")
    except Exception:
        hook = None
    mod = types.ModuleType("antenv.axon_hooks")
    mod.get_axon_ntff_profile_hook = lambda: hook
    mod.set_axon_ntff_profile_hook = lambda h: None
    try:
        import antenv

        antenv.axon_hooks = mod
    except ImportError:
        pass
    sys.modules["antenv.axon_hooks"] = mod


_ensure_axon_hooks_shim()

# Problem constants (hardcoded per contract; kernel.py must be self-contained)
G, W_SZ, H = 1024, 16, 16
B = 8192
F = G * W_SZ  # 16384 input features = output features (H == W_SZ)
N_CORES = 8
B_LOC = B // N_CORES  # 1024 batch rows per core

P = 128          # partitions
GROUPS_PER_SG = 128 // W_SZ   # 8 groups per 128x128 supergroup
N_SG = G // GROUPS_PER_SG     # 128 supergroups
SG_PER_BLK = 8                # supergroups per column block
N_BLK = N_SG // SG_PER_BLK    # 16 column blocks of 1024 columns
BLK_COLS = SG_PER_BLK * P     # 1024
BT = B_LOC // P               # 8 batch tiles per core

_cached = {}

# experiment knobs (bench only; defaults are the shipping config)
CONFIG = {
    "out_engine": "scalar",  # sync | scalar  (which HWDGE ring issues stores)
    "split_x": 1,            # pieces per 4MB x-block DMA
    "x_bufs": 3,
    "o_bufs": 3,
    "relu_mix": "alt",       # alt | act | dve
    "mm_dtype": "f32",       # f32 | f32r  (PE matmul input dtype)
    "pair_blks": 1,          # 1: pair column blocks -> 1MB stores, 8KB runs
}


def _build_program():
    """Build the (single-core SPMD) bass program once per process."""
    key = tuple(sorted(CONFIG.items()))
    if key in _cached:
        return _cached[key]

    f32 = mybir.dt.float32
    mdt = mybir.dt.float32r if CONFIG["mm_dtype"] == "f32r" else f32
    nc = bacc.Bacc("TRN2", debug=False, target_bir_lowering=False)

    xt_d = nc.dram_tensor("xt", (N_BLK, P, SG_PER_BLK * B_LOC), f32,
                          kind="ExternalInput")
    # compact weights: [jj, w, sg, h] (1 MB)
    wc_d = nc.dram_tensor("wc", (GROUPS_PER_SG, W_SZ, N_SG, H), f32,
                          kind="ExternalInput")
    out_d = nc.dram_tensor("out", (B_LOC, F), f32, kind="ExternalOutput")

    xt_ap = xt_d.ap()
    wc_ap = wc_d.ap()
    out_ap = out_d.ap()

    relu = mybir.ActivationFunctionType.Relu

    out_dma = nc.scalar if CONFIG["out_engine"] == "scalar" else nc.sync

    with tile.TileContext(nc) as tc:
        with (
            tc.tile_pool(name="wpool", bufs=1) as wpool,
            tc.tile_pool(name="xpool", bufs=CONFIG["x_bufs"]) as xpool,
            tc.tile_pool(name="opool", bufs=CONFIG["o_bufs"]) as opool,
            tc.tile_pool(name="pspool", bufs=8, space=bass.MemorySpace.PSUM) as pspool,
        ):
            # Build the resident block-diagonal weight tile once. Layout
            # groups each jj's data contiguously so the expansion DMA writes
            # one 8KB run per partition:
            #   wt_all[i, jj*2048 + sg*16 + h] = W[8*sg+jj, w, h]  (i = 16jj+w)
            # The matmul rhs for supergroup sg reads it back with a strided
            # 3-D AP whose (jj, h) enumeration equals output column o=16jj+h.
            wt_all = wpool.tile([P, N_SG * P], f32)
            third = (N_SG * P) // 3
            nc.vector.memset(wt_all[:, 0:third], 0.0)
            nc.scalar.memzero(wt_all[:, third:2 * third])
            nc.gpsimd.memset(wt_all[:, 2 * third:], 0.0)
            blk2 = N_SG * H  # 2048
            for jj in range(GROUPS_PER_SG):
                out_dma.dma_start(
                    wt_all[16 * jj:16 * jj + 16, jj * blk2:(jj + 1) * blk2],
                    wc_ap[jj],
                )
            wt_rhs = wt_all[:].rearrange("p (jj sg h) -> p jj sg h",
                                         jj=GROUPS_PER_SG, h=H)

            def compute_halves(xt_t, blk, bt, ot, o_off):
                for half in range(2):
                    ps = pspool.tile([P, 512], f32)
                    for q in range(4):
                        j = half * 4 + q
                        sg = blk * SG_PER_BLK + j
                        lhsT = xt_t[:, j * B_LOC + bt * P:
                                    j * B_LOC + bt * P + P]
                        rhs = wt_rhs[:, :, sg, :]
                        if mdt is not f32:
                            lhsT = lhsT.bitcast(mdt)
                            rhs = rhs.bitcast(mdt)
                        nc.tensor.matmul(ps[:, q * P:(q + 1) * P],
                                         lhsT, rhs,
                                         start=True, stop=True)
                    dst = ot[:, o_off + half * 512:o_off + (half + 1) * 512]
                    mix = CONFIG["relu_mix"]
                    use_act = (mix == "act" or
                               (mix == "alt" and (bt * 2 + half) % 2 == 0))
                    if use_act:
                        nc.scalar.activation(dst, ps[:], relu)
                    else:
                        nc.vector.tensor_scalar_max(dst, ps[:], 0.0)

            def load_x(blk):
                xt_t = xpool.tile([P, SG_PER_BLK * B_LOC], f32)
                nsp = CONFIG["split_x"]
                piece = (SG_PER_BLK * B_LOC) // nsp
                for sp in range(nsp):
                    nc.sync.dma_start(
                        xt_t[:, sp * piece:(sp + 1) * piece],
                        xt_ap[blk, :, sp * piece:(sp + 1) * piece],
                    )
                return xt_t

            if CONFIG["pair_blks"]:
                for pair in range(N_BLK // 2):
                    xts = [load_x(pair * 2), load_x(pair * 2 + 1)]
                    for bt in range(BT):
                        ot = opool.tile([P, 2 * BLK_COLS], f32)
                        for u in range(2):
                            compute_halves(xts[u], pair * 2 + u, bt, ot,
                                           u * BLK_COLS)
                        out_dma.dma_start(
                            out_ap[bt * P:(bt + 1) * P,
                                   pair * 2 * BLK_COLS:
                                   (pair + 1) * 2 * BLK_COLS],
                            ot[:],
                        )
            else:
                for blk in range(N_BLK):
                    xt_t = load_x(blk)
                    for bt in range(BT):
                        ot = opool.tile([P, BLK_COLS], f32)
                        compute_halves(xt_t, blk, bt, ot, 0)
                        out_dma.dma_start(
                            out_ap[bt * P:(bt + 1) * P,
                                   blk * BLK_COLS:(blk + 1) * BLK_COLS],
                            ot[:],
                        )

    nc.compile()
    _cached[key] = nc
    return nc


def _prep_w(W: np.ndarray) -> np.ndarray:
    """Compact weights reordered to [jj, w, sg, h] for on-chip expansion."""
    Wr = np.ascontiguousarray(W, dtype=np.float32).reshape(
        N_SG, GROUPS_PER_SG, W_SZ, H)
    return np.ascontiguousarray(Wr.transpose(1, 2, 0, 3))


def _prep_x_shard(xs: np.ndarray) -> np.ndarray:
    """Relayout one (1024, 16384) shard to (16, 128, 8*1024).

    xt[blk, p, j*1024 + b] = xs[b, blk*1024 + j*128 + p]
    """
    x4 = xs.reshape(B_LOC, N_BLK, SG_PER_BLK, P)          # b, blk, j, p
    xt = np.ascontiguousarray(x4.transpose(1, 3, 2, 0))    # blk, p, j, b
    return xt.reshape(N_BLK, P, SG_PER_BLK * B_LOC)


# Debug/benchmark knobs (used by test.py only; harness leaves defaults)
TRACE = False
TRACE_CORES = None  # e.g. [0] or list(range(8))
LAST_RESULTS = None


def kernel(x: np.ndarray, W: np.ndarray) -> np.ndarray:
    global LAST_RESULTS
    assert x.shape == (B, F) and W.shape == (G, W_SZ, H)
    x = np.ascontiguousarray(x, dtype=np.float32)

    wc = _prep_w(W)
    in_maps = []
    for s in range(N_CORES):
        xs = x[s * B_LOC:(s + 1) * B_LOC]
        in_maps.append({"xt": _prep_x_shard(xs), "wc": wc})

    nc = _build_program()
    kwargs = {}
    if TRACE:
        kwargs = {"trace": True, "trace_cores": TRACE_CORES}
    res = bass_utils.run_bass_kernel_spmd(nc, in_maps,
                                          core_ids=list(range(N_CORES)),
                                          **kwargs)
    LAST_RESULTS = res
    out = np.concatenate([r["out"] for r in res.results], axis=0)
    return out


# revision 36
# speedup vs baseline: 1.0287x; 1.0287x over previous
"""Block-diagonal grouped GEMM (BlockDense) for Trainium2, 8 NeuronCores.

Problem: x:(8192, 16384) f32, W:(1024, 16, 16) f32
         out[b, g*16+h] = relu(sum_w x[b, g*16+w] * W[g, w, h])

Strategy:
  - Data-parallel shard of the batch dim across 8 cores (1024 rows each).
  - Host relayouts each x shard so features sit on SBUF partitions
    (the PE contracts along partitions); 8 groups are packed into one
    128x128 block-diagonal weight supergroup so the full PE array is used.
  - Per core: for each of 16 column blocks (1024 cols = 8 supergroups):
    DMA x-block + W-block, 64 fp32 matmuls (stationary = xT tile),
    relu PSUM->SBUF on alternating Scalar/Vector engines, DMA out.
"""

import sys

import numpy as np

import concourse.bass as bass
import concourse.mybir as mybir
import concourse.tile as tile
from concourse import bacc, bass_utils
from concourse.tile_rust import add_dep_helper


def _ensure_axon_hooks_shim():
    """The bare agent image lacks antenv.axon_hooks; bass_utils imports it
    when trace=True under axon. Provide a working shim (ctypes NTFF hook if
    the axon .so supports it, else None -> tracing is skipped gracefully)."""
    try:
        import antenv.axon_hooks  # noqa: F401
        return
    except ImportError:
        pass
    import types

    hook = None
    try:
        from trn_agent_boot.trn_boot import _ntff_profile_via_ctypes

        hook = _ntff_profile_via_ctypes("/opt/axon/libaxon_pjrt.so")
    except Exception:
        hook = None
    mod = types.ModuleType("antenv.axon_hooks")
    mod.get_axon_ntff_profile_hook = lambda: hook
    mod.set_axon_ntff_profile_hook = lambda h: None
    try:
        import antenv

        antenv.axon_hooks = mod
    except ImportError:
        pass
    sys.modules["antenv.axon_hooks"] = mod


_ensure_axon_hooks_shim()

# Problem constants (hardcoded per contract; kernel.py must be self-contained)
G, W_SZ, H = 1024, 16, 16
B = 8192
F = G * W_SZ  # 16384 input features = output features (H == W_SZ)
N_CORES = 8
B_LOC = B // N_CORES  # 1024 batch rows per core

P = 128          # partitions
GROUPS_PER_SG = 128 // W_SZ   # 8 groups per 128x128 supergroup
N_SG = G // GROUPS_PER_SG     # 128 supergroups
SG_PER_BLK = 8                # supergroups per column block
N_BLK = N_SG // SG_PER_BLK    # 16 column blocks of 1024 columns
BLK_COLS = SG_PER_BLK * P     # 1024
BT = B_LOC // P               # 8 batch tiles per core

_cached = {}

# experiment knobs (bench only; defaults are the shipping config)
CONFIG = {
    "out_engine": "scalar",  # sync | scalar  (which HWDGE ring issues stores)
    "split_x": 1,            # pieces per 4MB x-block DMA
    "x_bufs": 3,
    "o_bufs": 3,
    "relu_mix": "alt",       # alt | act | dve
    "mm_dtype": "f32",       # f32 | f32r  (PE matmul input dtype)
    "pair_blks": 1,          # 1: pair column blocks -> 1MB stores, 8KB runs
    "serial_x": 1,           # 1: chain x loads so they complete in order
}


def _build_program():
    """Build the (single-core SPMD) bass program once per process."""
    key = tuple(sorted(CONFIG.items()))
    if key in _cached:
        return _cached[key]

    f32 = mybir.dt.float32
    mdt = mybir.dt.float32r if CONFIG["mm_dtype"] == "f32r" else f32
    nc = bacc.Bacc("TRN2", debug=False, target_bir_lowering=False)

    xt_d = nc.dram_tensor("xt", (N_BLK, P, SG_PER_BLK * B_LOC), f32,
                          kind="ExternalInput")
    # compact weights: [jj, w, sg, h] (1 MB)
    wc_d = nc.dram_tensor("wc", (GROUPS_PER_SG, W_SZ, N_SG, H), f32,
                          kind="ExternalInput")
    out_d = nc.dram_tensor("out", (B_LOC, F), f32, kind="ExternalOutput")

    xt_ap = xt_d.ap()
    wc_ap = wc_d.ap()
    out_ap = out_d.ap()

    relu = mybir.ActivationFunctionType.Relu

    out_dma = nc.scalar if CONFIG["out_engine"] == "scalar" else nc.sync

    with tile.TileContext(nc) as tc:
        with (
            tc.tile_pool(name="wpool", bufs=1) as wpool,
            tc.tile_pool(name="xpool", bufs=CONFIG["x_bufs"]) as xpool,
            tc.tile_pool(name="opool", bufs=CONFIG["o_bufs"]) as opool,
            tc.tile_pool(name="pspool", bufs=8, space=bass.MemorySpace.PSUM) as pspool,
        ):
            # Build the resident block-diagonal weight tile once. Layout
            # groups each jj's data contiguously so the expansion DMA writes
            # one 8KB run per partition:
            #   wt_all[i, jj*2048 + sg*16 + h] = W[8*sg+jj, w, h]  (i = 16jj+w)
            # The matmul rhs for supergroup sg reads it back with a strided
            # 3-D AP whose (jj, h) enumeration equals output column o=16jj+h.
            wt_all = wpool.tile([P, N_SG * P], f32)
            third = (N_SG * P) // 3
            nc.vector.memset(wt_all[:, 0:third], 0.0)
            nc.scalar.memzero(wt_all[:, third:2 * third])
            nc.gpsimd.memset(wt_all[:, 2 * third:], 0.0)
            blk2 = N_SG * H  # 2048
            for jj in range(GROUPS_PER_SG):
                out_dma.dma_start(
                    wt_all[16 * jj:16 * jj + 16, jj * blk2:(jj + 1) * blk2],
                    wc_ap[jj],
                )
            wt_rhs = wt_all[:].rearrange("p (jj sg h) -> p jj sg h",
                                         jj=GROUPS_PER_SG, h=H)

            def compute_halves(xt_t, blk, bt, ot, o_off):
                for half in range(2):
                    ps = pspool.tile([P, 512], f32)
                    for q in range(4):
                        j = half * 4 + q
                        sg = blk * SG_PER_BLK + j
                        lhsT = xt_t[:, j * B_LOC + bt * P:
                                    j * B_LOC + bt * P + P]
                        rhs = wt_rhs[:, :, sg, :]
                        if mdt is not f32:
                            lhsT = lhsT.bitcast(mdt)
                            rhs = rhs.bitcast(mdt)
                        nc.tensor.matmul(ps[:, q * P:(q + 1) * P],
                                         lhsT, rhs,
                                         start=True, stop=True)
                    dst = ot[:, o_off + half * 512:o_off + (half + 1) * 512]
                    mix = CONFIG["relu_mix"]
                    use_act = (mix == "act" or
                               (mix == "alt" and (bt * 2 + half) % 2 == 0))
                    if use_act:
                        nc.scalar.activation(dst, ps[:], relu)
                    else:
                        nc.vector.tensor_scalar_max(dst, ps[:], 0.0)

            prev_load = [None]

            def load_x(blk):
                xt_t = xpool.tile([P, SG_PER_BLK * B_LOC], f32)
                nsp = CONFIG["split_x"]
                piece = (SG_PER_BLK * B_LOC) // nsp
                for sp in range(nsp):
                    di = nc.sync.dma_start(
                        xt_t[:, sp * piece:(sp + 1) * piece],
                        xt_ap[blk, :, sp * piece:(sp + 1) * piece],
                    )
                    if CONFIG["serial_x"]:
                        if prev_load[0] is not None:
                            add_dep_helper(di.ins, prev_load[0],
                                           reason="serialize x loads")
                        prev_load[0] = di.ins
                return xt_t

            if CONFIG["pair_blks"]:
                for pair in range(N_BLK // 2):
                    xts = [load_x(pair * 2), load_x(pair * 2 + 1)]
                    for bt in range(BT):
                        ot = opool.tile([P, 2 * BLK_COLS], f32)
                        for u in range(2):
                            compute_halves(xts[u], pair * 2 + u, bt, ot,
                                           u * BLK_COLS)
                        out_dma.dma_start(
                            out_ap[bt * P:(bt + 1) * P,
                                   pair * 2 * BLK_COLS:
                                   (pair + 1) * 2 * BLK_COLS],
                            ot[:],
                        )
            else:
                for blk in range(N_BLK):
                    xt_t = load_x(blk)
                    for bt in range(BT):
                        ot = opool.tile([P, BLK_COLS], f32)
                        compute_halves(xt_t, blk, bt, ot, 0)
                        out_dma.dma_start(
                            out_ap[bt * P:(bt + 1) * P,
                                   blk * BLK_COLS:(blk + 1) * BLK_COLS],
                            ot[:],
                        )

    nc.compile()
    _cached[key] = nc
    return nc


def _prep_w(W: np.ndarray) -> np.ndarray:
    """Compact weights reordered to [jj, w, sg, h] for on-chip expansion."""
    Wr = np.ascontiguousarray(W, dtype=np.float32).reshape(
        N_SG, GROUPS_PER_SG, W_SZ, H)
    return np.ascontiguousarray(Wr.transpose(1, 2, 0, 3))


def _prep_x_shard(xs: np.ndarray) -> np.ndarray:
    """Relayout one (1024, 16384) shard to (16, 128, 8*1024).

    xt[blk, p, j*1024 + b] = xs[b, blk*1024 + j*128 + p]
    """
    x4 = xs.reshape(B_LOC, N_BLK, SG_PER_BLK, P)          # b, blk, j, p
    xt = np.ascontiguousarray(x4.transpose(1, 3, 2, 0))    # blk, p, j, b
    return xt.reshape(N_BLK, P, SG_PER_BLK * B_LOC)


# Debug/benchmark knobs (used by test.py only; harness leaves defaults)
TRACE = False
TRACE_CORES = None  # e.g. [0] or list(range(8))
LAST_RESULTS = None


def kernel(x: np.ndarray, W: np.ndarray) -> np.ndarray:
    global LAST_RESULTS
    assert x.shape == (B, F) and W.shape == (G, W_SZ, H)
    x = np.ascontiguousarray(x, dtype=np.float32)

    wc = _prep_w(W)
    in_maps = []
    for s in range(N_CORES):
        xs = x[s * B_LOC:(s + 1) * B_LOC]
        in_maps.append({"xt": _prep_x_shard(xs), "wc": wc})

    nc = _build_program()
    kwargs = {}
    if TRACE:
        kwargs = {"trace": True, "trace_cores": TRACE_CORES}
    res = bass_utils.run_bass_kernel_spmd(nc, in_maps,
                                          core_ids=list(range(N_CORES)),
                                          **kwargs)
    LAST_RESULTS = res
    out = np.concatenate([r["out"] for r in res.results], axis=0)
    return out


# revision 38
# speedup vs baseline: 1.0566x; 1.0271x over previous
"""Block-diagonal grouped GEMM (BlockDense) for Trainium2, 8 NeuronCores.

Problem: x:(8192, 16384) f32, W:(1024, 16, 16) f32
         out[b, g*16+h] = relu(sum_w x[b, g*16+w] * W[g, w, h])

Strategy:
  - Data-parallel shard of the batch dim across 8 cores (1024 rows each).
  - Host relayouts each x shard so features sit on SBUF partitions
    (the PE contracts along partitions); 8 groups are packed into one
    128x128 block-diagonal weight supergroup so the full PE array is used.
  - Per core: for each of 16 column blocks (1024 cols = 8 supergroups):
    DMA x-block + W-block, 64 fp32 matmuls (stationary = xT tile),
    relu PSUM->SBUF on alternating Scalar/Vector engines, DMA out.
"""

import sys

import numpy as np

import concourse.bass as bass
import concourse.mybir as mybir
import concourse.tile as tile
from concourse import bacc, bass_utils
from concourse.tile_rust import add_dep_helper


def _ensure_axon_hooks_shim():
    """The bare agent image lacks antenv.axon_hooks; bass_utils imports it
    when trace=True under axon. Provide a working shim (ctypes NTFF hook if
    the axon .so supports it, else None -> tracing is skipped gracefully)."""
    try:
        import antenv.axon_hooks  # noqa: F401
        return
    except ImportError:
        pass
    import types

    hook = None
    try:
        from trn_agent_boot.trn_boot import _ntff_profile_via_ctypes

        hook = _ntff_profile_via_ctypes("/opt/axon/libaxon_pjrt.so")
    except Exception:
        hook = None
    mod = types.ModuleType("antenv.axon_hooks")
    mod.get_axon_ntff_profile_hook = lambda: hook
    mod.set_axon_ntff_profile_hook = lambda h: None
    try:
        import antenv

        antenv.axon_hooks = mod
    except ImportError:
        pass
    sys.modules["antenv.axon_hooks"] = mod


_ensure_axon_hooks_shim()

# Problem constants (hardcoded per contract; kernel.py must be self-contained)
G, W_SZ, H = 1024, 16, 16
B = 8192
F = G * W_SZ  # 16384 input features = output features (H == W_SZ)
N_CORES = 8
B_LOC = B // N_CORES  # 1024 batch rows per core

P = 128          # partitions
GROUPS_PER_SG = 128 // W_SZ   # 8 groups per 128x128 supergroup
N_SG = G // GROUPS_PER_SG     # 128 supergroups
SG_PER_BLK = 8                # supergroups per column block
N_BLK = N_SG // SG_PER_BLK    # 16 column blocks of 1024 columns
BLK_COLS = SG_PER_BLK * P     # 1024
BT = B_LOC // P               # 8 batch tiles per core

_cached = {}

# experiment knobs (bench only; defaults are the shipping config)
CONFIG = {
    "out_engine": "scalar",  # sync | scalar  (which HWDGE ring issues stores)
    "split_x": 1,            # pieces per 4MB x-block DMA
    "x_bufs": 3,
    "o_bufs": 3,
    "relu_mix": "alt",       # alt | act | dve
    "mm_dtype": "f32",       # f32 | f32r  (PE matmul input dtype)
    "pair_blks": 1,          # 1: pair column blocks -> 1MB stores, 8KB runs
    "serial_x": 1,           # 1: chain x loads so they complete in order
}


def _build_program():
    """Build the (single-core SPMD) bass program once per process."""
    key = tuple(sorted(CONFIG.items()))
    if key in _cached:
        return _cached[key]

    f32 = mybir.dt.float32
    mdt = mybir.dt.float32r if CONFIG["mm_dtype"] == "f32r" else f32
    nc = bacc.Bacc("TRN2", debug=False, target_bir_lowering=False)

    xt_d = nc.dram_tensor("xt", (N_BLK, P, SG_PER_BLK * B_LOC), f32,
                          kind="ExternalInput")
    # compact weights: [jj, w, sg, h] (1 MB)
    wc_d = nc.dram_tensor("wc", (GROUPS_PER_SG, W_SZ, N_SG, H), f32,
                          kind="ExternalInput")
    out_d = nc.dram_tensor("out", (B_LOC, F), f32, kind="ExternalOutput")

    xt_ap = xt_d.ap()
    wc_ap = wc_d.ap()
    out_ap = out_d.ap()

    relu = mybir.ActivationFunctionType.Relu

    out_dma = nc.scalar if CONFIG["out_engine"] == "scalar" else nc.sync

    with tile.TileContext(nc) as tc:
        with (
            tc.tile_pool(name="wpool", bufs=1) as wpool,
            tc.tile_pool(name="xpool", bufs=CONFIG["x_bufs"]) as xpool,
            tc.tile_pool(name="opool", bufs=CONFIG["o_bufs"]) as opool,
            tc.tile_pool(name="pspool", bufs=8, space=bass.MemorySpace.PSUM) as pspool,
        ):
            # Build the resident block-diagonal weight tile once. Layout
            # groups each jj's data contiguously so the expansion DMA writes
            # one 8KB run per partition:
            #   wt_all[i, jj*2048 + sg*16 + h] = W[8*sg+jj, w, h]  (i = 16jj+w)
            # The matmul rhs for supergroup sg reads it back with a strided
            # 3-D AP whose (jj, h) enumeration equals output column o=16jj+h.
            wt_all = wpool.tile([P, N_SG * P], f32)
            blk2 = N_SG * H  # 2048
            # Per-jj memset then per-jj weight DMA: each DMA only waits on
            # its own column range, so the expansion pipelines instead of
            # stalling on one full-tile memset barrier.
            ms_engines = [nc.vector, nc.scalar, nc.gpsimd]
            for jj in range(GROUPS_PER_SG):
                eng = ms_engines[jj % 3]
                seg = wt_all[:, jj * blk2:(jj + 1) * blk2]
                if eng is nc.scalar:
                    eng.memzero(seg)
                else:
                    eng.memset(seg, 0.0)
                out_dma.dma_start(
                    wt_all[16 * jj:16 * jj + 16, jj * blk2:(jj + 1) * blk2],
                    wc_ap[jj],
                )
            wt_rhs = wt_all[:].rearrange("p (jj sg h) -> p jj sg h",
                                         jj=GROUPS_PER_SG, h=H)

            def compute_halves(xt_t, blk, bt, ot, o_off):
                for half in range(2):
                    ps = pspool.tile([P, 512], f32)
                    for q in range(4):
                        j = half * 4 + q
                        sg = blk * SG_PER_BLK + j
                        lhsT = xt_t[:, j * B_LOC + bt * P:
                                    j * B_LOC + bt * P + P]
                        rhs = wt_rhs[:, :, sg, :]
                        if mdt is not f32:
                            lhsT = lhsT.bitcast(mdt)
                            rhs = rhs.bitcast(mdt)
                        nc.tensor.matmul(ps[:, q * P:(q + 1) * P],
                                         lhsT, rhs,
                                         start=True, stop=True)
                    dst = ot[:, o_off + half * 512:o_off + (half + 1) * 512]
                    mix = CONFIG["relu_mix"]
                    use_act = (mix == "act" or
                               (mix == "alt" and (bt * 2 + half) % 2 == 0))
                    if use_act:
                        nc.scalar.activation(dst, ps[:], relu)
                    else:
                        nc.vector.tensor_scalar_max(dst, ps[:], 0.0)

            prev_load = [None]

            def load_x(blk):
                xt_t = xpool.tile([P, SG_PER_BLK * B_LOC], f32)
                # finer pieces for the first pair so compute starts sooner
                nsp = 2 if blk < 2 else CONFIG["split_x"]
                piece = (SG_PER_BLK * B_LOC) // nsp
                for sp in range(nsp):
                    di = nc.sync.dma_start(
                        xt_t[:, sp * piece:(sp + 1) * piece],
                        xt_ap[blk, :, sp * piece:(sp + 1) * piece],
                    )
                    if CONFIG["serial_x"]:
                        if prev_load[0] is not None:
                            add_dep_helper(di.ins, prev_load[0],
                                           reason="serialize x loads")
                        prev_load[0] = di.ins
                return xt_t

            if CONFIG["pair_blks"]:
                for pair in range(N_BLK // 2):
                    xts = [load_x(pair * 2), load_x(pair * 2 + 1)]
                    for bt in range(BT):
                        ot = opool.tile([P, 2 * BLK_COLS], f32)
                        for u in range(2):
                            compute_halves(xts[u], pair * 2 + u, bt, ot,
                                           u * BLK_COLS)
                        out_dma.dma_start(
                            out_ap[bt * P:(bt + 1) * P,
                                   pair * 2 * BLK_COLS:
                                   (pair + 1) * 2 * BLK_COLS],
                            ot[:],
                        )
            else:
                for blk in range(N_BLK):
                    xt_t = load_x(blk)
                    for bt in range(BT):
                        ot = opool.tile([P, BLK_COLS], f32)
                        compute_halves(xt_t, blk, bt, ot, 0)
                        out_dma.dma_start(
                            out_ap[bt * P:(bt + 1) * P,
                                   blk * BLK_COLS:(blk + 1) * BLK_COLS],
                            ot[:],
                        )

    nc.compile()
    _cached[key] = nc
    return nc


def _prep_w(W: np.ndarray) -> np.ndarray:
    """Compact weights reordered to [jj, w, sg, h] for on-chip expansion."""
    Wr = np.ascontiguousarray(W, dtype=np.float32).reshape(
        N_SG, GROUPS_PER_SG, W_SZ, H)
    return np.ascontiguousarray(Wr.transpose(1, 2, 0, 3))


def _prep_x_shard(xs: np.ndarray) -> np.ndarray:
    """Relayout one (1024, 16384) shard to (16, 128, 8*1024).

    xt[blk, p, j*1024 + b] = xs[b, blk*1024 + j*128 + p]
    """
    x4 = xs.reshape(B_LOC, N_BLK, SG_PER_BLK, P)          # b, blk, j, p
    xt = np.ascontiguousarray(x4.transpose(1, 3, 2, 0))    # blk, p, j, b
    return xt.reshape(N_BLK, P, SG_PER_BLK * B_LOC)


# Debug/benchmark knobs (used by test.py only; harness leaves defaults)
TRACE = False
TRACE_CORES = None  # e.g. [0] or list(range(8))
LAST_RESULTS = None


def kernel(x: np.ndarray, W: np.ndarray) -> np.ndarray:
    global LAST_RESULTS
    assert x.shape == (B, F) and W.shape == (G, W_SZ, H)
    x = np.ascontiguousarray(x, dtype=np.float32)

    wc = _prep_w(W)
    in_maps = []
    for s in range(N_CORES):
        xs = x[s * B_LOC:(s + 1) * B_LOC]
        in_maps.append({"xt": _prep_x_shard(xs), "wc": wc})

    nc = _build_program()
    kwargs = {}
    if TRACE:
        kwargs = {"trace": True, "trace_cores": TRACE_CORES}
    res = bass_utils.run_bass_kernel_spmd(nc, in_maps,
                                          core_ids=list(range(N_CORES)),
                                          **kwargs)
    LAST_RESULTS = res
    out = np.concatenate([r["out"] for r in res.results], axis=0)
    return out
